# revision 9
# baseline (speedup 1.0000x reference)
"""Trainium2 Bass kernel: backward pass of a stride-2 SAME conv2d (3x3, C=256).

Computes (grad_in, grad_wt, grad_bias) from (grad_output, inputs, kernels),
matching reference.backward_conv.

Distribution: data-parallel over batch (32 -> 4 per core, 8 NeuronCores).
  - grad_in is batch-sharded; host concatenates shards.
  - grad_wt / grad_bias contract over batch: per-core partials + on-device
    AllReduce; host reads core 0's copy.

Per-core math (Bs=4):
  grad_in[b,2i+py,2j+px,ci] = sum over taps (ky,kx) of parity (py,px):
      go[b,i+di,j+dj,:] @ W'[ky,kx]   (zero at boundary)
  where W'[ky,kx][co,ci] = kernels[2-ky,2-kx,ci,co],
  taps with ky in {0,2} feed py=0 (di = ky/2 - 1), ky==1 feeds py=1 (di=0);
  same for kx/px/dj.
  grad_wt[ky,kx,ci,co] = sum_{b,i,j} inpP[b,2i+ky,2j+kx,ci] * go[b,i,j,co]
      (inpP = inputs zero-padded on the high edge)
  grad_bias[co] = sum_{b,i,j} go[b,i,j,co]  (ones-vector matmul, shares rhs)

All matmuls in bf16 (fp32 PSUM accumulation); host pre-transposes/pads all
operands so the device does zero layout work.
"""

import numpy as np
import ml_dtypes

import concourse.bacc as bacc
import concourse.mybir as mybir
from concourse import tile
from concourse.bass_utils import run_bass_kernel_spmd

BF16 = mybir.dt.bfloat16
F32 = mybir.dt.float32

N_CORES = 8
B, HO, WO, C = 32, 28, 28, 256
H, W = 56, 56
BS = B // N_CORES                 # 4
PIX = BS * HO * WO                # 3136
GOT_COLS = BS * 29 * 29           # 3364
NTAP = 9
GW_ROWS = NTAP * 2 * 128          # 2304
AR_ROWS = GW_ROWS + 8             # 2312 (8 identical bias rows)

# tap -> (parity class, di, dj)
TAPS = []
for ky in range(3):
    for kx in range(3):
        py, di = ((0, ky // 2 - 1) if ky != 1 else (1, 0))
        px, dj = ((0, kx // 2 - 1) if kx != 1 else (1, 0))
        TAPS.append((ky, kx, py * 2 + px, di, dj))

_CACHED = {}


def _build_nc():
    nc = bacc.Bacc("TRN2", target_bir_lowering=False, debug=False,
                   num_devices=N_CORES)

    goT = nc.declare_dram_parameter("goT", [2, 128, GOT_COLS], BF16, isOutput=False)
    goP = nc.declare_dram_parameter("goP", [PIX, C], BF16, isOutput=False)
    inpP = nc.declare_dram_parameter("inpP", [BS, 58, 58, C], BF16, isOutput=False)
    wgi = nc.declare_dram_parameter("wgi", [128, NTAP * 4 * 128], BF16, isOutput=False)

    gi_out = nc.declare_dram_parameter("gi_out", [4, 2, 128, PIX], F32, isOutput=True)
    gw_out = nc.declare_dram_parameter("gw_out", [AR_ROWS, C], F32, isOutput=True)

    gw_bounce = nc.dram_tensor("gw_bounce", [AR_ROWS, C], F32)
    gw_red = nc.dram_tensor("gw_red", [AR_ROWS, C], F32, addr_space="Shared")

    # inpP viewed as [b, row_par, row_half, col_half, col_par, c]
    inp_v = inpP.ap().rearrange("b (rh rp) (wc wp) c -> b rp rh wc wp c",
                                rp=2, wp=2)
    # goP viewed per (b): [b, p=(i_sub,j):112, ig, c]
    gop_v = goP.ap().rearrange("(b ig p) c -> b p ig c", b=BS, ig=7, p=112)

    with tile.TileContext(nc) as tc:
        with (
            tc.tile_pool(name="const", bufs=1) as cpool,
            tc.tile_pool(name="rhs", bufs=1) as rhs_pool,
            tc.tile_pool(name="lhs", bufs=6) as lhs_pool,
            tc.tile_pool(name="copy", bufs=4) as copy_pool,
            tc.tile_pool(name="psum", bufs=8, space="PSUM") as psum_pool,
        ):
            ones = cpool.tile([128, 8], BF16, tag="ones")
            nc.gpsimd.memset(ones[:, :], 1.0)

            # goP resident: per b one [112, 7, C] tile, partition p=(j, i_sub)
            rhs_go = []
            for b in range(BS):
                t = rhs_pool.tile([112, 7, C], BF16, tag=f"rhsgo{b}",
                                  name=f"rhsgo{b}")
                nc.sync.dma_start(out=t[:, :, :], in_=gop_v[b])
                rhs_go.append(t)

            # ---------- phase A: grad_wt partials + bias ----------
            for ky in range(3):
                rp, rh0 = ky & 1, (1 if ky == 2 else 0)
                ps_gw = [psum_pool.tile([128, C], F32, tag="ps", name=f"psgw{ky}_{i}")
                         for i in range(6)]  # idx = kx*2 + ci_ck
                ps_bias = (psum_pool.tile([8, C], F32, tag="ps", name="psbias")
                           if ky == 1 else None)
                for b in range(BS):
                    for ig in range(7):
                        rh = rh0 + 4 * ig
                        # T1: cols 2j+{0,1}, partition (i_sub:4, j:28)
                        t1 = lhs_pool.tile([112, 2, C], BF16, tag="t1")
                        nc.sync.dma_start(
                            out=t1[:, :, :],
                            in_=inp_v[b, rp, rh:rh + 4, 0:28, :, :])
                        # T2: cols 2j+2  (= wc j+1, wp 0)
                        t2 = lhs_pool.tile([112, C], BF16, tag="t2")
                        nc.sync.dma_start(
                            out=t2[:, :],
                            in_=inp_v[b, rp, rh:rh + 4, 1:29, 0, :])
                        first = b == 0 and ig == 0
                        last = b == BS - 1 and ig == 6
                        for kx in range(3):
                            for ck in range(2):
                                lsl = (t1[:, kx, ck * 128:(ck + 1) * 128]
                                       if kx < 2 else
                                       t2[:, ck * 128:(ck + 1) * 128])
                                nc.tensor.matmul(
                                    ps_gw[kx * 2 + ck][:, :], lsl,
                                    rhs_go[b][:, ig, :],
                                    start=first, stop=last)
                        if ky == 1:
                            nc.tensor.matmul(
                                ps_bias[:, :], ones[0:112, :],
                                rhs_go[b][:, ig, :],
                                start=first, stop=last)
                for kx in range(3):
                    for ck in range(2):
                        sb = copy_pool.tile([128, C], F32, tag="cp")
                        nc.vector.tensor_copy(sb[:, :], ps_gw[kx * 2 + ck][:, :])
                        row = ((ky * 3 + kx) * 2 + ck) * 128
                        nc.sync.dma_start(
                            out=gw_bounce[row:row + 128, :], in_=sb[:, :])
                if ky == 1:
                    sbb = copy_pool.tile([8, C], F32, tag="cpb")
                    nc.vector.tensor_copy(sbb[:, :], ps_bias[:, :])
                    nc.sync.dma_start(
                        out=gw_bounce[GW_ROWS:GW_ROWS + 8, :], in_=sbb[:, :])

            nc.gpsimd.collective_compute(
                "AllReduce", mybir.AluOpType.add,
                replica_groups=[list(range(N_CORES))],
                ins=[gw_bounce[:, :]], outs=[gw_red[:, :]],
            )
            nc.sync.dma_start(out=gw_out[:, :], in_=gw_red[:, :])

            # ---------- phase B: grad_in ----------
            goT_sb = []
            for ck in range(2):
                t = rhs_pool.tile([128, GOT_COLS], BF16, tag=f"goT{ck}",
                                  name=f"goTsb{ck}")
                nc.sync.dma_start(out=t[:, :], in_=goT[ck, :, :])
                goT_sb.append(t)
            wgi_sb = rhs_pool.tile([128, NTAP * 4 * 128], BF16, tag="wgi")
            nc.sync.dma_start(out=wgi_sb[:, :], in_=wgi[:, :])
            goT_v = [t[:, :].rearrange("p (b u v) -> p b u v", b=BS, u=29, v=29)
                     for t in goT_sb]

            for pc in range(4):
                taps = [t for t in TAPS if t[2] == pc]
                for ci_ck in range(2):
                    for b in range(BS):
                        for ih in range(2):
                            ps = psum_pool.tile([128, 392], F32, tag="ps")
                            nmm = len(taps) * 2
                            mi = 0
                            for (ky, kx, _, di, dj) in taps:
                                ti = ky * 3 + kx
                                u0 = 1 + ih * 14 + di
                                v0 = 1 + dj
                                for co_ck in range(2):
                                    rhs = goT_v[co_ck][:, b, u0:u0 + 14,
                                                       v0:v0 + 28]
                                    idx = (ti * 4 + co_ck * 2 + ci_ck) * 128
                                    nc.tensor.matmul(
                                        ps[:, :],
                                        wgi_sb[:, idx:idx + 128],
                                        rhs,
                                        start=(mi == 0), stop=(mi == nmm - 1))
                                    mi += 1
                            sb = copy_pool.tile([128, 392], F32, tag="cp2")
                            nc.vector.tensor_copy(sb[:, :], ps[:, :])
                            off = b * 784 + ih * 392
                            nc.sync.dma_start(
                                out=gi_out[pc, ci_ck, :, off:off + 392],
                                in_=sb[:, :])

    nc.compile()
    return nc


def _prep_core_inputs(go_s, inp_s, wgi_host):
    """go_s [4,28,28,256] f32, inp_s [4,56,56,256] f32 -> in_map dict."""
    bf = ml_dtypes.bfloat16
    go_bf = go_s.astype(bf)

    goT = np.zeros((BS, 29, 29, C), dtype=bf)
    goT[:, 1:, 1:, :] = go_bf
    goT = np.ascontiguousarray(
        goT.transpose(3, 0, 1, 2).reshape(2, 128, GOT_COLS))

    goP = np.ascontiguousarray(go_bf.reshape(PIX, C))

    inpP = np.zeros((BS, 58, 58, C), dtype=bf)
    inpP[:, :56, :56, :] = inp_s.astype(bf)

    return {"goT": goT, "goP": goP, "inpP": inpP, "wgi": wgi_host}


def kernel(grad_output, inputs, kernels):
    if "nc" not in _CACHED:
        _CACHED["nc"] = _build_nc()
    nc = _CACHED["nc"]

    bf = ml_dtypes.bfloat16
    # wgi[co_in, ((ky*3+kx)*4 + co_ck*2 + ci_ck)*128 + ci_in]
    #   = kernels[2-ky, 2-kx, ci, co]
    kr = np.ascontiguousarray(kernels[::-1, ::-1]).astype(bf)
    kr = kr.reshape(3, 3, 2, 128, 2, 128)        # ky kx ci_ck ci_in co_ck co_in
    wgi_host = np.ascontiguousarray(
        kr.transpose(5, 0, 1, 4, 2, 3).reshape(128, NTAP * 4 * 128))

    grad_output = np.asarray(grad_output)
    inputs = np.asarray(inputs)
    in_maps = []
    for c in range(N_CORES):
        sl = slice(c * BS, (c + 1) * BS)
        in_maps.append(_prep_core_inputs(grad_output[sl], inputs[sl], wgi_host))

    res = run_bass_kernel_spmd(nc, in_maps, list(range(N_CORES)))
    _CACHED["last_res"] = res

    gi_parts = []
    for c in range(N_CORES):
        dev = res.results[c]["gi_out"]                   # [4,2,128,3136]
        dev = dev.reshape(2, 2, 2, 128, BS, HO, WO)      # py px ck ci b i j
        gi_parts.append(
            dev.transpose(4, 5, 0, 6, 1, 2, 3).reshape(BS, H, W, C))
    grad_in = np.ascontiguousarray(
        np.concatenate(gi_parts, axis=0)).astype(np.float32)

    gw_full = res.results[0]["gw_out"]                   # [2312,256] summed
    grad_wt = np.ascontiguousarray(
        gw_full[:GW_ROWS].reshape(3, 3, 256, C)).astype(np.float32)
    grad_bias = np.ascontiguousarray(gw_full[GW_ROWS]).astype(np.float32)

    return grad_in, grad_wt, grad_bias
